# revision 81
# baseline (speedup 1.0000x reference)
import os
import sys

for _p in ("/opt/trn_rl_repo", "/root/.axon_site/_ro/trn_rl_repo"):
    if os.path.isdir(_p) and _p not in sys.path:
        sys.path.insert(0, _p)

import numpy as np
import ml_dtypes

BF16 = ml_dtypes.bfloat16

HEADS, D = 12, 64
WINDOW, SHIFT = 16, 1
SCALE = D ** -0.5
B, N, DIM = 2, 2049, 768
INNER = HEADS * D  # 768
TAUG = 258  # CLS slot + tok1/dummy slot + 256 block tokens
NCORES = 8
KT = DIM // 128  # 6
VW = HEADS * 65  # 780: per-head 64 v-cols + ones-col at 65h+64

STARTS = [2, 258, 514, 770, 1026, 1282, 1538, 1794]
ENDS = [258, 514, 770, 1026, 1282, 1538, 1794, 2049]

LEGACY_ATTN = True

_NC_CACHE = {}


def _build_nc():
    import concourse.bass as bass
    import concourse.bacc as bacc
    import concourse.mybir as mybir
    import concourse.tile as tile

    f32 = mybir.dt.float32
    bf16 = mybir.dt.bfloat16
    Exp = mybir.ActivationFunctionType.Exp
    Copy = mybir.ActivationFunctionType.Copy

    nc = bacc.Bacc(None, target_bir_lowering=False)

    xT_ext = nc.declare_dram_parameter("xaT", (B, KT, 128, TAUG), bf16, isOutput=False)
    # q/k weights as 12 column-slices [128, KT*128] in order q0,k0,q1,k1,...;
    # col-block k of slice = w[128k:128(k+1), cols].
    wqk_ext = nc.declare_dram_parameter("wqk", (12, 128, KT * 128), bf16, isOutput=False)
    wv_ext = nc.declare_dram_parameter("wv", (KT, 128, VW), bf16, isOutput=False)
    wout_ext = nc.declare_dram_parameter("w_out", (INNER, DIM), bf16, isOutput=False)
    bout_ext = nc.declare_dram_parameter("b_out", (128, DIM), bf16, isOutput=False)
    mask_ext = nc.declare_dram_parameter("mask4", (128, 512), bf16, isOutput=False)
    id_ext = nc.declare_dram_parameter("ident", (128, 128), bf16, isOutput=False)
    vcr_ext = nc.declare_dram_parameter("vc_rep", (B, 1, VW), bf16, isOutput=False)
    # zero-split CLS-key stationaries: col 2g has head 2g's k_cls in rows
    # 0:64 (zeros below); col 2g+1 has head 2g+1's k_cls in rows 64:128.
    kcls_ext = nc.declare_dram_parameter("kcls", (B, 128, 12), bf16, isOutput=False)
    out_ext = nc.declare_dram_parameter("out_tokens", (B, 256, DIM), bf16, isOutput=True)
    cls_ext = nc.declare_dram_parameter("cls_part", (B, HEADS, VW), f32, isOutput=True)

    with tile.TileContext(nc) as tc:
        with (
            tc.tile_pool(name="wpool", bufs=1) as wp,
            tc.tile_pool(name="fpool", bufs=2) as fp,
            tc.tile_pool(name="spool", bufs=6) as sp,
            tc.tile_pool(name="psA", bufs=2, space="PSUM") as psA,
            tc.tile_pool(name="psS", bufs=2, space="PSUM") as psS,
        ):
            # ---- input DMAs in compute order ----
            xT = [[None] * KT for _ in range(B)]
            for k in range(KT):
                t = fp.tile([128, TAUG], bf16, tag=f"xT0_{k}", name=f"xT0_{k}")
                nc.sync.dma_start(t[:], xT_ext[0, k])
                xT[0][k] = t[:]
            vcr, kcls_t = [None] * B, [None] * B
            vcr[0] = fp.tile([1, VW], bf16, tag="vcr0", name="vcr0")
            nc.sync.dma_start(vcr[0][:], vcr_ext[0])
            kcls_t[0] = wp.tile([128, 12], bf16, tag="kcls0", name="kcls0")
            nc.sync.dma_start(kcls_t[0][:], kcls_ext[0])
            wqk_t = []
            for s in range(12):
                t = wp.tile([128, KT * 128], bf16, tag=f"wqk{s}")
                nc.sync.dma_start(t[:], wqk_ext[s])
                wqk_t.append(t)
            wv_t = []
            for k in range(KT):
                t = wp.tile([128, VW], bf16, tag=f"wv{k}")
                nc.sync.dma_start(t[:], wv_ext[k])
                wv_t.append(t)
            mask4 = wp.tile([128, 512], bf16, tag="mask4")
            nc.sync.dma_start(mask4[:], mask_ext[:])
            ident = wp.tile([128, 128], bf16, tag="ident")
            nc.sync.dma_start(ident[:], id_ext[:])
            for k in range(KT):
                t = fp.tile([128, TAUG], bf16, tag=f"xT1_{k}", name=f"xT1_{k}")
                nc.sync.dma_start(t[:], xT_ext[1, k])
                xT[1][k] = t[:]
            vcr[1] = fp.tile([1, VW], bf16, tag="vcr1", name="vcr1")
            nc.sync.dma_start(vcr[1][:], vcr_ext[1])
            kcls_t[1] = wp.tile([128, 12], bf16, tag="kcls1", name="kcls1")
            nc.sync.dma_start(kcls_t[1][:], kcls_ext[1])
            wo_t = []
            for k in range(KT):
                t = wp.tile([128, DIM], bf16, tag=f"wo{k}")
                nc.sync.dma_start(t[:], wout_ext[k * 128:(k + 1) * 128, :])
                wo_t.append(t)
            bias_full = wp.tile([128, DIM], bf16, tag="bias_full")
            nc.sync.dma_start(bias_full[:], bout_ext[:])

            qT = [[None] * 6 for _ in range(B)]
            kTt = [[None] * 6 for _ in range(B)]
            vs = [[None, None] for _ in range(B)]
            att_s = [[fp.tile([128, INNER], bf16, tag=f"att{b}_{s}", name=f"att{b}_{s}")
                      for s in range(2)] for b in range(B)]
            aT = [[fp.tile([128, 256], bf16, tag=f"aT{b}_{i}", name=f"aT{b}_{i}")
                   for i in range(KT)] for b in range(B)]
            clspt = psS.tile([128, 512], f32, tag="cls", bufs=1, name="clspt")
            clsp = [clspt[:, 256 * b:256 * b + 256] for b in range(B)]

            def qk_unit(b, j, g, pref, dst):
                # j=0 -> q slice, j=1 -> k slice
                w = wqk_t[2 * g + j]
                ps = psA.tile([128, 512], f32, tag="big", name="ps")
                for k in range(KT):
                    nc.tensor.matmul(ps[:, 0:TAUG],
                                     w[:, 128 * k: 128 * (k + 1)],
                                     xT[b][k], start=(k == 0), stop=(k == KT - 1))
                t = fp.tile([128, TAUG], bf16, tag=f"{pref}T{b}_{g}", name=f"{pref}T{b}_{g}")
                nc.vector.tensor_copy(t[:], ps[:, 0:TAUG])
                dst[b][g] = t

            def v_unit(b, ti):
                vt = fp.tile([128, VW], bf16, tag=f"v{b}_{ti}", name=f"v{b}_{ti}")
                for c0, cw in ((0, 512), (512, VW - 512)):
                    pv = psA.tile([128, 512], f32, tag="big", name="pv")
                    for k in range(KT):
                        nc.tensor.matmul(pv[:, 0:cw],
                                         xT[b][k][:, 2 + 128 * ti: 2 + 128 * (ti + 1)],
                                         wv_t[k][:, c0: c0 + cw],
                                         start=(k == 0), stop=(k == KT - 1))
                    nc.vector.tensor_copy(vt[:, c0:c0 + cw], pv[:, 0:cw])
                nc.vector.memset(vt[:, 64:VW:65], 1.0)
                vs[b][ti] = vt

            def attn_head_legacy(b, h):
                g, p0 = h // 2, 64 * (h % 2)
                kk, qq = kTt[b][g], qT[b][g]
                hps = psS.tile([128, 512], f32, tag="hqk", name="hps")
                ecp2 = psS.tile([128, 512], f32, tag="ecp", bufs=1, name="ecp2")
                for half in range(2):
                    nc.tensor.matmul(ecp2[0:1, 256 * half:256 * half + 256],
                                     kk[p0:p0 + 64, 0:1], qq[p0:p0 + 64, 2:TAUG],
                                     start=True, stop=True, skip_group_check=True)
                ecr = sp.tile([1, 512], bf16, tag="ecr", name="ecr")
                nc.scalar.activation(ecr[:], ecp2[0:1, 0:512], Exp, scale=SCALE)
                # ecr col space: token index - 2
                for s in range(2):
                    q0 = 2 + 128 * s
                    nc.tensor.matmul(clsp[b][:, 12 * s + h:12 * s + h + 1],
                                     kk[p0:p0 + 64, q0:q0 + 128],
                                     qq[p0:p0 + 64, 0:1], start=True, stop=True,
                                     skip_group_check=True)
                pst = psS.tile([128, 256], f32, tag="hav", name="pst")
                for s in range(2):
                    q0 = 2 + 128 * s
                    nc.tensor.matmul(pst[:, 128 * s:128 * s + 128],
                                     kk[p0:p0 + 64, q0:q0 + 128],
                                     qq[p0:p0 + 64, q0:q0 + 128], start=True, stop=True,
                                     skip_group_check=True)
                prob = sp.tile([128, 256], bf16, tag="prob", name="prob")
                nc.scalar.activation(prob[:], pst[:], Exp, scale=SCALE)
                nc.vector.tensor_mul(prob[:], prob[:], mask4[:, 0:256])
                for s in range(2):
                    pc = TAUG + 65 * s
                    nc.tensor.matmul(hps[:, pc:pc + 65], prob[:, 128 * s:128 * s + 128],
                                     vs[b][s][:, 65 * h:65 * h + 65],
                                     start=True, stop=False, skip_group_check=True)
                    nc.tensor.matmul(hps[:, pc:pc + 65], ecr[0:1, 128 * s:128 * s + 128],
                                     vcr[b][0:1, 65 * h:65 * h + 65],
                                     start=False, stop=True, skip_group_check=True)
                rec = sp.tile([128, 2], f32, tag="rec", name="rec")
                nc.vector.reciprocal(rec[:], hps[:, TAUG + 64:TAUG + 130:65])
                for s in range(2):
                    pc = TAUG + 65 * s
                    nc.scalar.activation(att_s[b][s][:, 64 * h:64 * h + 64],
                                         hps[:, pc:pc + 64], Copy, scale=rec[:, s:s + 1])
                # HAM keep-warm: dummy matmuls into an unused, never-read
                # region of the persistent clspt bank; they run in the PE
                # idle window while Scalar normalizes this head.
                for _ in range(4):
                    nc.tensor.matmul(clspt[:, 384:512], ident[:, 0:128], ident[:],
                                     start=True, stop=True, skip_group_check=True)

            # ---- attention head-pair, split into front (QK/exp) and back
            # (AV/normalize) halves for software pipelining ----
            pair_state = {}

            def pair_front(b, g):
                kk, qq = kTt[b][g], qT[b][g]
                hq = [psS.tile([128, 256], f32, tag="hqk", name=f"hqk{h01}")
                      for h01 in range(2)]
                for h01 in range(2):
                    p0 = 64 * h01
                    h = 2 * g + h01
                    for s in range(2):
                        q0 = 2 + 128 * s
                        nc.tensor.matmul(hq[h01][:, 128 * s:128 * s + 128],
                                         kk[p0:p0 + 64, q0:q0 + 128],
                                         qq[p0:p0 + 64, q0:q0 + 128],
                                         start=True, stop=True, skip_group_check=True)
                        nc.tensor.matmul(clsp[b][:, 12 * s + h:12 * s + h + 1],
                                         kk[p0:p0 + 64, q0:q0 + 128],
                                         qq[p0:p0 + 64, 0:1], start=True, stop=True,
                                         skip_group_check=True)
                ecp = psS.tile([128, 512], f32, tag="ecp", bufs=1, name="ecp")
                for h01 in range(2):
                    p0 = 64 * h01
                    nc.tensor.matmul(ecp[0:1, 256 * h01:256 * h01 + 256],
                                     kk[p0:p0 + 64, 0:1],
                                     qq[p0:p0 + 64, 2:TAUG], start=True, stop=True,
                                     skip_group_check=True)
                ecr = sp.tile([1, 512], bf16, tag="ecr", name="ecr")
                nc.scalar.activation(ecr[:], ecp[0:1, :], Exp, scale=SCALE)
                prob2 = sp.tile([128, 512], bf16, tag="prob", name="prob2")
                for h01 in range(2):
                    c0 = 256 * h01
                    nc.scalar.activation(prob2[:, c0:c0 + 256], hq[h01][:],
                                         Exp, scale=SCALE)
                    nc.vector.tensor_mul(prob2[:, c0:c0 + 256], prob2[:, c0:c0 + 256],
                                         mask4[:, c0:c0 + 256])
                pair_state[(b, g)] = (prob2, ecr)

            def pair_back(b, g):
                prob2, ecr = pair_state.pop((b, g))
                hav = [psS.tile([128, 130], f32, tag="hav", name=f"hav{h01}")
                       for h01 in range(2)]
                for h01 in range(2):
                    h = 2 * g + h01
                    for s in range(2):
                        pc = 65 * s
                        nc.tensor.matmul(hav[h01][:, pc:pc + 65],
                                         prob2[:, 256 * h01 + 128 * s:256 * h01 + 128 * s + 128],
                                         vs[b][s][:, 65 * h:65 * h + 65],
                                         start=True, stop=False, skip_group_check=True)
                        nc.tensor.matmul(hav[h01][:, pc:pc + 65],
                                         ecr[0:1, 256 * h01 + 128 * s:256 * h01 + 128 * s + 128],
                                         vcr[b][0:1, 65 * h:65 * h + 65],
                                         start=False, stop=True, skip_group_check=True)
                for h01 in range(2):
                    h = 2 * g + h01
                    for s in range(2):
                        pc = 65 * s
                        rec = sp.tile([128, 1], f32, tag="rec", name="rec")
                        nc.vector.reciprocal(rec[:], hav[h01][:, pc + 64:pc + 65])
                        dst = att_s[b][s][:, 64 * h:64 * h + 64]
                        nc.scalar.activation(dst, hav[h01][:, pc:pc + 64], Copy,
                                             scale=rec[:, 0:1])

            def cls_unit(b):
                eccs = []
                for s in range(2):
                    E = sp.tile([128, HEADS], bf16, tag="ECC", name="E")
                    nc.scalar.activation(E[:], clsp[b][:, 12 * s:12 * (s + 1)], Exp, scale=SCALE)
                    eccs.append(E)
                clsA = psS.tile([128, 512], f32, tag="hqk", name="clsA")
                clsB = psS.tile([128, 512], f32, tag="hqk", name="clsB")
                for s in range(2):
                    nc.tensor.matmul(clsA[0:HEADS, :], eccs[s][:], vs[b][s][:, 0:512],
                                     start=(s == 0), stop=(s == 1), skip_group_check=True)
                    nc.tensor.matmul(clsB[0:HEADS, 0:VW - 512], eccs[s][:], vs[b][s][:, 512:VW],
                                     start=(s == 0), stop=(s == 1), skip_group_check=True)
                cls_sb = sp.tile([HEADS, VW], f32, tag="clssb", name="cls_sb")
                nc.vector.tensor_copy(cls_sb[:, 0:512], clsA[0:HEADS, :])
                nc.vector.tensor_copy(cls_sb[:, 512:VW], clsB[0:HEADS, 0:VW - 512])
                nc.sync.dma_start(cls_ext[b], cls_sb[:])

            def tr_unit(b, i):
                for s in range(2):
                    pt = psA.tile([128, 128], bf16, tag="big", name="pt")
                    nc.tensor.transpose(pt[:], att_s[b][s][:, 128 * i:128 * (i + 1)], ident[:])
                    nc.vector.tensor_copy(aT[b][i][:, 128 * s:128 * (s + 1)], pt[:])

            def proj_unit(b, ti):
                ot = fp.tile([128, DIM], bf16, tag=f"ot{b}_{ti}", name=f"ot{b}_{ti}")
                for c0, cw in ((0, 512), (512, 256)):
                    po = psA.tile([128, 512], f32, tag="big", name="po")
                    for i in range(KT):
                        nc.tensor.matmul(po[:, 0:cw], aT[b][i][:, 128 * ti:128 * (ti + 1)],
                                         wo_t[i][:, c0:c0 + cw],
                                         start=(i == 0), stop=(i == KT - 1))
                    nc.vector.tensor_add(ot[:, c0:c0 + cw], po[:, 0:cw],
                                         bias_full[:, c0:c0 + cw])
                    nc.sync.dma_start(
                        out_ext[b, 128 * ti:128 * (ti + 1), c0:c0 + cw],
                        ot[:, c0:c0 + cw])

            # ---- HAM warm-up: dummy matmuls on the first-arriving x tile
            # fill the DMA-wait window so phase 1 starts at full PE clock ----
            wps = psA.tile([128, 512], f32, tag="big", name="wps")
            for _ in range(40):
                nc.tensor.matmul(wps[:, 0:128], xT[0][0][:, 0:128],
                                 xT[0][0][:, 0:128], start=True, stop=True,
                                 skip_group_check=True)

            # ---- phase 1: QKV(b0) ----
            for g in range(6):
                qk_unit(0, 0, g, "q", qT)
                qk_unit(0, 1, g, "k", kTt)
            v_unit(0, 0)
            v_unit(0, 1)

            # ---- phase 2: attention(b0), QKV(b1) interleaved ----
            b1_units = ([lambda g=g, j=j: qk_unit(1, j, g, "qk"[j], (qT, kTt)[j])
                         for g in range(6) for j in range(2)]
                        + [lambda ti=ti: v_unit(1, ti) for ti in range(2)])
            for g in range(6):
                if LEGACY_ATTN:
                    attn_head_legacy(0, 2 * g)
                    attn_head_legacy(0, 2 * g + 1)
                else:
                    pair_front(0, g)
                    pair_back(0, g)
                tr_unit(0, g)
                for f in b1_units[14 * g // 6:14 * (g + 1) // 6]:
                    f()
            for f in b1_units[14:]:
                f()

            # ---- phase 3: attention(b1), b0 tail interleaved ----
            b0_tail = [lambda: cls_unit(0),
                       lambda: proj_unit(0, 0),
                       lambda: proj_unit(0, 1)]
            for g in range(6):
                if LEGACY_ATTN:
                    attn_head_legacy(1, 2 * g)
                    attn_head_legacy(1, 2 * g + 1)
                else:
                    pair_front(1, g)
                    pair_back(1, g)
                tr_unit(1, g)
                if g < 3:
                    b0_tail[g]()

            # ---- phase 4: tail for b1 ----
            cls_unit(1)
            proj_unit(1, 0)
            proj_unit(1, 1)

    nc.compile()
    return nc


def _get_nc():
    if "nc" not in _NC_CACHE:
        _NC_CACHE["nc"] = _build_nc()
    return _NC_CACHE["nc"]


def _make_masks(core):
    start = STARTS[core]
    masks = np.zeros((2, 128, 128), dtype=np.float32)
    for s in range(2):
        g = start + s * 128 + np.arange(128)
        real = g < 2049
        blk = (g - 2) // 16
        same = (blk[:, None] == blk[None, :]) & real[:, None] & real[None, :]
        masks[s] = same.astype(np.float32)
    return masks


def _v65(row768):
    out = np.zeros(VW, dtype=np.float64)
    for h in range(HEADS):
        out[65 * h:65 * h + 64] = row768[64 * h:64 * h + 64]
        out[65 * h + 64] = 1.0
    return out


def _kslab(w):
    # [768, C] -> [128, KT*C] with col-block k = w[128k:128(k+1), :]
    c = w.shape[1]
    return w.reshape(KT, 128, c).transpose(1, 0, 2).reshape(128, KT * c)


def _make_in_maps(x, w_qkv, w_out, b_out):
    x = np.asarray(x, dtype=np.float32)
    w_qkv = np.asarray(w_qkv, dtype=np.float32)
    w_out = np.asarray(w_out, dtype=np.float32)
    b_out = np.asarray(b_out, dtype=np.float32)

    wqk = np.zeros((12, 128, KT * 128), dtype=np.float32)
    for g in range(6):
        wqk[2 * g] = _kslab(w_qkv[:, 128 * g:128 * (g + 1)])
        wqk[2 * g + 1] = _kslab(w_qkv[:, 768 + 128 * g:768 + 128 * (g + 1)])
    wqk_b = wqk.astype(BF16)
    wv65 = np.zeros((DIM, VW), dtype=np.float32)
    for h in range(HEADS):
        wv65[:, 65 * h:65 * h + 64] = w_qkv[:, 1536 + 64 * h:1536 + 64 * h + 64]
    wv_b = wv65.reshape(KT, 128, VW).astype(BF16)
    wo_b = w_out.astype(BF16)

    w_v = w_qkv[:, 1536:]
    w_k = w_qkv[:, 768:1536]
    vcls = x[:, 0, :] @ w_v
    kcls_f = x[:, 0, :] @ w_k

    vcrs = np.zeros((B, 1, VW), dtype=np.float32)
    kcls = np.zeros((B, 128, 12), dtype=np.float32)
    for b in range(B):
        vcrs[b, 0] = _v65(vcls[b])
        for g in range(6):
            kcls[b, 0:64, 2 * g] = kcls_f[b, 64 * 2 * g:64 * 2 * g + 64]
            kcls[b, 64:128, 2 * g + 1] = kcls_f[b, 64 * (2 * g + 1):64 * (2 * g + 1) + 64]

    ident = np.eye(128, dtype=BF16)
    bias_tiled = np.tile(b_out.reshape(1, DIM), (128, 1)).astype(BF16)
    in_maps = []
    for c in range(NCORES):
        xaug = np.zeros((B, TAUG, DIM), dtype=np.float32)
        xaug[:, 0, :] = x[:, 0, :]
        if c == 0:
            xaug[:, 1, :] = x[:, 1, :]
        L = ENDS[c] - STARTS[c]
        xaug[:, 2:2 + L, :] = x[:, STARTS[c]:ENDS[c], :]
        xaT = xaug.transpose(0, 2, 1).reshape(B, KT, 128, TAUG)
        m = _make_masks(c)
        mask4 = np.concatenate([m[0], m[1], m[0], m[1]], axis=1)
        in_maps.append({
            "xaT": xaT.astype(BF16),
            "wqk": wqk_b,
            "wv": wv_b,
            "w_out": wo_b,
            "b_out": bias_tiled,
            "mask4": mask4.astype(BF16),
            "ident": ident,
            "vc_rep": vcrs.astype(BF16),
            "kcls": kcls.astype(BF16),
        })
    return in_maps


def kernel(x, w_qkv, w_out, b_out):
    x_f = np.asarray(x, dtype=np.float64)
    w_qkv_f = np.asarray(w_qkv, dtype=np.float64)
    w_out_f = np.asarray(w_out, dtype=np.float64)
    b_out_f = np.asarray(b_out, dtype=np.float64)
    in_maps = _make_in_maps(x, w_qkv, w_out, b_out)

    from concourse.bass_utils import run_bass_kernel_spmd

    nc = _get_nc()
    res = run_bass_kernel_spmd(nc, in_maps, core_ids=list(range(NCORES))).results

    out = np.empty((B, N, DIM), dtype=np.float32)
    for c in range(NCORES):
        L = ENDS[c] - STARTS[c]
        out[:, STARTS[c]:ENDS[c], :] = res[c]["out_tokens"][:, :L, :]

    # CLS / tok1 rows: device partial sums over block keys + host-computed
    # contributions of the cls/tok1 keys themselves.
    w_q = w_qkv_f[:, 0:768]
    w_k = w_qkv_f[:, 768:1536]
    w_v = w_qkv_f[:, 1536:]
    for b in range(B):
        acc = np.zeros((HEADS, VW), dtype=np.float64)
        for c in range(NCORES):
            acc += res[c]["cls_part"][b].astype(np.float64)
            acc[:, 64::65] -= 256 - (ENDS[c] - STARTS[c])
        qc = x_f[b, 0] @ w_q
        qt = x_f[b, 1] @ w_q
        kc = x_f[b, 0] @ w_k
        kt = x_f[b, 1] @ w_k
        v65c = _v65(x_f[b, 0] @ w_v)
        v65t = _v65(x_f[b, 1] @ w_v)
        t1x = np.zeros((2 * HEADS, VW), dtype=np.float64)
        for h in range(HEADS):
            sl = slice(64 * h, 64 * h + 64)
            for j, qv in ((0, qc), (1, qt)):
                ec = np.exp(SCALE * np.dot(kc[sl], qv[sl]))
                et = np.exp(SCALE * np.dot(kt[sl], qv[sl]))
                t1x[2 * h + j] = ec * v65c + et * v65t
        acc += t1x[0::2]
        cls_flat = np.empty(INNER, dtype=np.float64)
        t1_flat = np.empty(INNER, dtype=np.float64)
        for h in range(HEADS):
            cls_flat[64 * h:64 * h + 64] = acc[h, 65 * h:65 * h + 64] / acc[h, 65 * h + 64]
            t1_flat[64 * h:64 * h + 64] = (t1x[2 * h + 1, 65 * h:65 * h + 64]
                                           / t1x[2 * h + 1, 65 * h + 64])
        out[b, 0, :] = (cls_flat @ w_out_f + b_out_f).astype(np.float32)
        out[b, 1, :] = (t1_flat @ w_out_f + b_out_f).astype(np.float32)
    return out


# revision 82
# speedup vs baseline: 1.0412x; 1.0412x over previous
import os
import sys

for _p in ("/opt/trn_rl_repo", "/root/.axon_site/_ro/trn_rl_repo"):
    if os.path.isdir(_p) and _p not in sys.path:
        sys.path.insert(0, _p)

import numpy as np
import ml_dtypes

BF16 = ml_dtypes.bfloat16

HEADS, D = 12, 64
WINDOW, SHIFT = 16, 1
SCALE = D ** -0.5
B, N, DIM = 2, 2049, 768
INNER = HEADS * D  # 768
TAUG = 258  # CLS slot + tok1/dummy slot + 256 block tokens
NCORES = 8
KT = DIM // 128  # 6
VW = HEADS * 65  # 780: per-head 64 v-cols + ones-col at 65h+64

STARTS = [2, 258, 514, 770, 1026, 1282, 1538, 1794]
ENDS = [258, 514, 770, 1026, 1282, 1538, 1794, 2049]

LEGACY_ATTN = True

_NC_CACHE = {}


def _build_nc():
    import concourse.bass as bass
    import concourse.bacc as bacc
    import concourse.mybir as mybir
    import concourse.tile as tile

    f32 = mybir.dt.float32
    bf16 = mybir.dt.bfloat16
    Exp = mybir.ActivationFunctionType.Exp
    Copy = mybir.ActivationFunctionType.Copy

    nc = bacc.Bacc(None, target_bir_lowering=False)

    xT_ext = nc.declare_dram_parameter("xaT", (B, KT, 128, TAUG), bf16, isOutput=False)
    # q/k weights as 12 column-slices [128, KT*128] in order q0,k0,q1,k1,...;
    # col-block k of slice = w[128k:128(k+1), cols].
    wqk_ext = nc.declare_dram_parameter("wqk", (12, 128, KT * 128), bf16, isOutput=False)
    wv_ext = nc.declare_dram_parameter("wv", (KT, 128, VW), bf16, isOutput=False)
    wout_ext = nc.declare_dram_parameter("w_out", (INNER, DIM), bf16, isOutput=False)
    bout_ext = nc.declare_dram_parameter("b_out", (128, DIM), bf16, isOutput=False)
    mask_ext = nc.declare_dram_parameter("mask4", (128, 512), bf16, isOutput=False)
    id_ext = nc.declare_dram_parameter("ident", (128, 128), bf16, isOutput=False)
    vcr_ext = nc.declare_dram_parameter("vc_rep", (B, 1, VW), bf16, isOutput=False)
    # zero-split CLS-key stationaries: col 2g has head 2g's k_cls in rows
    # 0:64 (zeros below); col 2g+1 has head 2g+1's k_cls in rows 64:128.
    kcls_ext = nc.declare_dram_parameter("kcls", (B, 128, 12), bf16, isOutput=False)
    out_ext = nc.declare_dram_parameter("out_tokens", (B, 256, DIM), bf16, isOutput=True)
    cls_ext = nc.declare_dram_parameter("cls_part", (B, HEADS, VW), f32, isOutput=True)

    with tile.TileContext(nc) as tc:
        with (
            tc.tile_pool(name="wpool", bufs=1) as wp,
            tc.tile_pool(name="fpool", bufs=2) as fp,
            tc.tile_pool(name="spool", bufs=6) as sp,
            tc.tile_pool(name="psA", bufs=2, space="PSUM") as psA,
            tc.tile_pool(name="psS", bufs=2, space="PSUM") as psS,
        ):
            # ---- input DMAs in compute order ----
            xT = [[None] * KT for _ in range(B)]
            for k in range(KT):
                t = fp.tile([128, TAUG], bf16, tag=f"xT0_{k}", name=f"xT0_{k}")
                nc.sync.dma_start(t[:], xT_ext[0, k])
                xT[0][k] = t[:]
            vcr, kcls_t = [None] * B, [None] * B
            vcr[0] = fp.tile([1, VW], bf16, tag="vcr0", name="vcr0")
            nc.sync.dma_start(vcr[0][:], vcr_ext[0])
            kcls_t[0] = wp.tile([128, 12], bf16, tag="kcls0", name="kcls0")
            nc.sync.dma_start(kcls_t[0][:], kcls_ext[0])
            wqk_t = []
            for s in range(12):
                t = wp.tile([128, KT * 128], bf16, tag=f"wqk{s}")
                nc.sync.dma_start(t[:], wqk_ext[s])
                wqk_t.append(t)
            wv_t = []
            for k in range(KT):
                t = wp.tile([128, VW], bf16, tag=f"wv{k}")
                nc.sync.dma_start(t[:], wv_ext[k])
                wv_t.append(t)
            mask4 = wp.tile([128, 512], bf16, tag="mask4")
            nc.sync.dma_start(mask4[:], mask_ext[:])
            ident = wp.tile([128, 128], bf16, tag="ident")
            nc.sync.dma_start(ident[:], id_ext[:])
            for k in range(KT):
                t = fp.tile([128, TAUG], bf16, tag=f"xT1_{k}", name=f"xT1_{k}")
                nc.sync.dma_start(t[:], xT_ext[1, k])
                xT[1][k] = t[:]
            vcr[1] = fp.tile([1, VW], bf16, tag="vcr1", name="vcr1")
            nc.sync.dma_start(vcr[1][:], vcr_ext[1])
            kcls_t[1] = wp.tile([128, 12], bf16, tag="kcls1", name="kcls1")
            nc.sync.dma_start(kcls_t[1][:], kcls_ext[1])
            wo_t = []
            for k in range(KT):
                t = wp.tile([128, DIM], bf16, tag=f"wo{k}")
                nc.sync.dma_start(t[:], wout_ext[k * 128:(k + 1) * 128, :])
                wo_t.append(t)
            bias_full = wp.tile([128, DIM], bf16, tag="bias_full")
            nc.sync.dma_start(bias_full[:], bout_ext[:])

            qT = [[None] * 6 for _ in range(B)]
            kTt = [[None] * 6 for _ in range(B)]
            vs = [[None, None] for _ in range(B)]
            att_s = [[fp.tile([128, INNER], bf16, tag=f"att{b}_{s}", name=f"att{b}_{s}")
                      for s in range(2)] for b in range(B)]
            aT = [[fp.tile([128, 256], bf16, tag=f"aT{b}_{i}", name=f"aT{b}_{i}")
                   for i in range(KT)] for b in range(B)]
            clspt = psS.tile([128, 512], f32, tag="cls", bufs=1, name="clspt")
            clsp = [clspt[:, 256 * b:256 * b + 256] for b in range(B)]

            def qk_unit(b, j, g, pref, dst):
                # j=0 -> q slice, j=1 -> k slice
                w = wqk_t[2 * g + j]
                ps = psA.tile([128, 512], f32, tag="big", name="ps")
                for k in range(KT):
                    nc.tensor.matmul(ps[:, 0:TAUG],
                                     w[:, 128 * k: 128 * (k + 1)],
                                     xT[b][k], start=(k == 0), stop=(k == KT - 1))
                t = fp.tile([128, TAUG], bf16, tag=f"{pref}T{b}_{g}", name=f"{pref}T{b}_{g}")
                nc.vector.tensor_copy(t[:], ps[:, 0:TAUG])
                dst[b][g] = t

            def v_unit(b, ti):
                vt = fp.tile([128, VW], bf16, tag=f"v{b}_{ti}", name=f"v{b}_{ti}")
                for c0, cw in ((0, 512), (512, VW - 512)):
                    pv = psA.tile([128, 512], f32, tag="big", name="pv")
                    for k in range(KT):
                        nc.tensor.matmul(pv[:, 0:cw],
                                         xT[b][k][:, 2 + 128 * ti: 2 + 128 * (ti + 1)],
                                         wv_t[k][:, c0: c0 + cw],
                                         start=(k == 0), stop=(k == KT - 1))
                    nc.vector.tensor_copy(vt[:, c0:c0 + cw], pv[:, 0:cw])
                nc.vector.memset(vt[:, 64:VW:65], 1.0)
                vs[b][ti] = vt

            def attn_head_legacy(b, h):
                g, p0 = h // 2, 64 * (h % 2)
                kk, qq = kTt[b][g], qT[b][g]
                hps = psS.tile([128, 512], f32, tag="hqk", name="hps")
                ecp2 = psS.tile([128, 512], f32, tag="ecp", bufs=1, name="ecp2")
                for half in range(2):
                    nc.tensor.matmul(ecp2[0:1, 256 * half:256 * half + 256],
                                     kk[p0:p0 + 64, 0:1], qq[p0:p0 + 64, 2:TAUG],
                                     start=True, stop=True, skip_group_check=True)
                ecr = sp.tile([1, 512], bf16, tag="ecr", name="ecr")
                nc.scalar.activation(ecr[:], ecp2[0:1, 0:512], Exp, scale=SCALE)
                # ecr col space: token index - 2
                for s in range(2):
                    q0 = 2 + 128 * s
                    nc.tensor.matmul(clsp[b][:, 12 * s + h:12 * s + h + 1],
                                     kk[p0:p0 + 64, q0:q0 + 128],
                                     qq[p0:p0 + 64, 0:1], start=True, stop=True,
                                     skip_group_check=True)
                pst = psS.tile([128, 256], f32, tag="hav", name="pst")
                for s in range(2):
                    q0 = 2 + 128 * s
                    nc.tensor.matmul(pst[:, 128 * s:128 * s + 128],
                                     kk[p0:p0 + 64, q0:q0 + 128],
                                     qq[p0:p0 + 64, q0:q0 + 128], start=True, stop=True,
                                     skip_group_check=True)
                prob = sp.tile([128, 256], bf16, tag="prob", name="prob")
                nc.scalar.activation(prob[:], pst[:], Exp, scale=SCALE)
                nc.vector.tensor_mul(prob[:], prob[:], mask4[:, 0:256])
                for s in range(2):
                    pc = TAUG + 65 * s
                    nc.tensor.matmul(hps[:, pc:pc + 65], prob[:, 128 * s:128 * s + 128],
                                     vs[b][s][:, 65 * h:65 * h + 65],
                                     start=True, stop=False, skip_group_check=True)
                    nc.tensor.matmul(hps[:, pc:pc + 65], ecr[0:1, 128 * s:128 * s + 128],
                                     vcr[b][0:1, 65 * h:65 * h + 65],
                                     start=False, stop=True, skip_group_check=True)
                rec = sp.tile([128, 2], f32, tag="rec", name="rec")
                nc.vector.reciprocal(rec[:], hps[:, TAUG + 64:TAUG + 130:65])
                for s in range(2):
                    pc = TAUG + 65 * s
                    nc.scalar.activation(att_s[b][s][:, 64 * h:64 * h + 64],
                                         hps[:, pc:pc + 64], Copy, scale=rec[:, s:s + 1])

            # ---- attention head-pair, split into front (QK/exp) and back
            # (AV/normalize) halves for software pipelining ----
            pair_state = {}

            def pair_front(b, g):
                kk, qq = kTt[b][g], qT[b][g]
                hq = [psS.tile([128, 256], f32, tag="hqk", name=f"hqk{h01}")
                      for h01 in range(2)]
                for h01 in range(2):
                    p0 = 64 * h01
                    h = 2 * g + h01
                    for s in range(2):
                        q0 = 2 + 128 * s
                        nc.tensor.matmul(hq[h01][:, 128 * s:128 * s + 128],
                                         kk[p0:p0 + 64, q0:q0 + 128],
                                         qq[p0:p0 + 64, q0:q0 + 128],
                                         start=True, stop=True, skip_group_check=True)
                        nc.tensor.matmul(clsp[b][:, 12 * s + h:12 * s + h + 1],
                                         kk[p0:p0 + 64, q0:q0 + 128],
                                         qq[p0:p0 + 64, 0:1], start=True, stop=True,
                                         skip_group_check=True)
                ecp = psS.tile([128, 512], f32, tag="ecp", bufs=1, name="ecp")
                for h01 in range(2):
                    p0 = 64 * h01
                    nc.tensor.matmul(ecp[0:1, 256 * h01:256 * h01 + 256],
                                     kk[p0:p0 + 64, 0:1],
                                     qq[p0:p0 + 64, 2:TAUG], start=True, stop=True,
                                     skip_group_check=True)
                ecr = sp.tile([1, 512], bf16, tag="ecr", name="ecr")
                nc.scalar.activation(ecr[:], ecp[0:1, :], Exp, scale=SCALE)
                prob2 = sp.tile([128, 512], bf16, tag="prob", name="prob2")
                for h01 in range(2):
                    c0 = 256 * h01
                    nc.scalar.activation(prob2[:, c0:c0 + 256], hq[h01][:],
                                         Exp, scale=SCALE)
                    nc.vector.tensor_mul(prob2[:, c0:c0 + 256], prob2[:, c0:c0 + 256],
                                         mask4[:, c0:c0 + 256])
                pair_state[(b, g)] = (prob2, ecr)

            def pair_back(b, g):
                prob2, ecr = pair_state.pop((b, g))
                hav = [psS.tile([128, 130], f32, tag="hav", name=f"hav{h01}")
                       for h01 in range(2)]
                for h01 in range(2):
                    h = 2 * g + h01
                    for s in range(2):
                        pc = 65 * s
                        nc.tensor.matmul(hav[h01][:, pc:pc + 65],
                                         prob2[:, 256 * h01 + 128 * s:256 * h01 + 128 * s + 128],
                                         vs[b][s][:, 65 * h:65 * h + 65],
                                         start=True, stop=False, skip_group_check=True)
                        nc.tensor.matmul(hav[h01][:, pc:pc + 65],
                                         ecr[0:1, 256 * h01 + 128 * s:256 * h01 + 128 * s + 128],
                                         vcr[b][0:1, 65 * h:65 * h + 65],
                                         start=False, stop=True, skip_group_check=True)
                for h01 in range(2):
                    h = 2 * g + h01
                    for s in range(2):
                        pc = 65 * s
                        rec = sp.tile([128, 1], f32, tag="rec", name="rec")
                        nc.vector.reciprocal(rec[:], hav[h01][:, pc + 64:pc + 65])
                        dst = att_s[b][s][:, 64 * h:64 * h + 64]
                        nc.scalar.activation(dst, hav[h01][:, pc:pc + 64], Copy,
                                             scale=rec[:, 0:1])

            def cls_unit(b):
                eccs = []
                for s in range(2):
                    E = sp.tile([128, HEADS], bf16, tag="ECC", name="E")
                    nc.scalar.activation(E[:], clsp[b][:, 12 * s:12 * (s + 1)], Exp, scale=SCALE)
                    eccs.append(E)
                clsA = psS.tile([128, 512], f32, tag="hqk", name="clsA")
                clsB = psS.tile([128, 512], f32, tag="hqk", name="clsB")
                for s in range(2):
                    nc.tensor.matmul(clsA[0:HEADS, :], eccs[s][:], vs[b][s][:, 0:512],
                                     start=(s == 0), stop=(s == 1), skip_group_check=True)
                    nc.tensor.matmul(clsB[0:HEADS, 0:VW - 512], eccs[s][:], vs[b][s][:, 512:VW],
                                     start=(s == 0), stop=(s == 1), skip_group_check=True)
                cls_sb = sp.tile([HEADS, VW], f32, tag="clssb", name="cls_sb")
                nc.vector.tensor_copy(cls_sb[:, 0:512], clsA[0:HEADS, :])
                nc.vector.tensor_copy(cls_sb[:, 512:VW], clsB[0:HEADS, 0:VW - 512])
                nc.sync.dma_start(cls_ext[b], cls_sb[:])

            def tr_unit(b, i):
                for s in range(2):
                    pt = psA.tile([128, 128], bf16, tag="big", name="pt")
                    nc.tensor.transpose(pt[:], att_s[b][s][:, 128 * i:128 * (i + 1)], ident[:])
                    nc.vector.tensor_copy(aT[b][i][:, 128 * s:128 * (s + 1)], pt[:])

            def proj_unit(b, ti):
                ot = fp.tile([128, DIM], bf16, tag=f"ot{b}_{ti}", name=f"ot{b}_{ti}")
                for c0, cw in ((0, 512), (512, 256)):
                    po = psA.tile([128, 512], f32, tag="big", name="po")
                    for i in range(KT):
                        nc.tensor.matmul(po[:, 0:cw], aT[b][i][:, 128 * ti:128 * (ti + 1)],
                                         wo_t[i][:, c0:c0 + cw],
                                         start=(i == 0), stop=(i == KT - 1))
                    nc.vector.tensor_add(ot[:, c0:c0 + cw], po[:, 0:cw],
                                         bias_full[:, c0:c0 + cw])
                    nc.sync.dma_start(
                        out_ext[b, 128 * ti:128 * (ti + 1), c0:c0 + cw],
                        ot[:, c0:c0 + cw])

            # ---- HAM warm-up: dummy matmuls on the first-arriving x tile
            # fill the DMA-wait window so phase 1 starts at full PE clock ----
            wps = psA.tile([128, 512], f32, tag="big", name="wps")
            for _ in range(40):
                nc.tensor.matmul(wps[:, 0:128], xT[0][0][:, 0:128],
                                 xT[0][0][:, 0:128], start=True, stop=True,
                                 skip_group_check=True)

            # ---- phase 1: QKV(b0) ----
            for g in range(6):
                qk_unit(0, 0, g, "q", qT)
                qk_unit(0, 1, g, "k", kTt)
            v_unit(0, 0)
            v_unit(0, 1)

            # ---- phase 2: attention(b0), QKV(b1) interleaved ----
            b1_units = ([lambda g=g, j=j: qk_unit(1, j, g, "qk"[j], (qT, kTt)[j])
                         for g in range(6) for j in range(2)]
                        + [lambda ti=ti: v_unit(1, ti) for ti in range(2)])
            for g in range(6):
                if LEGACY_ATTN:
                    attn_head_legacy(0, 2 * g)
                    attn_head_legacy(0, 2 * g + 1)
                else:
                    pair_front(0, g)
                    pair_back(0, g)
                tr_unit(0, g)
                for f in b1_units[14 * g // 6:14 * (g + 1) // 6]:
                    f()
            for f in b1_units[14:]:
                f()

            # ---- phase 3: attention(b1), b0 tail interleaved ----
            b0_tail = [lambda: cls_unit(0),
                       lambda: proj_unit(0, 0),
                       lambda: proj_unit(0, 1)]
            for g in range(6):
                if LEGACY_ATTN:
                    attn_head_legacy(1, 2 * g)
                    attn_head_legacy(1, 2 * g + 1)
                else:
                    pair_front(1, g)
                    pair_back(1, g)
                tr_unit(1, g)
                if g < 3:
                    b0_tail[g]()

            # ---- phase 4: tail for b1 ----
            cls_unit(1)
            proj_unit(1, 0)
            proj_unit(1, 1)

    nc.compile()
    return nc


def _get_nc():
    if "nc" not in _NC_CACHE:
        _NC_CACHE["nc"] = _build_nc()
    return _NC_CACHE["nc"]


def _make_masks(core):
    start = STARTS[core]
    masks = np.zeros((2, 128, 128), dtype=np.float32)
    for s in range(2):
        g = start + s * 128 + np.arange(128)
        real = g < 2049
        blk = (g - 2) // 16
        same = (blk[:, None] == blk[None, :]) & real[:, None] & real[None, :]
        masks[s] = same.astype(np.float32)
    return masks


def _v65(row768):
    out = np.zeros(VW, dtype=np.float64)
    for h in range(HEADS):
        out[65 * h:65 * h + 64] = row768[64 * h:64 * h + 64]
        out[65 * h + 64] = 1.0
    return out


def _kslab(w):
    # [768, C] -> [128, KT*C] with col-block k = w[128k:128(k+1), :]
    c = w.shape[1]
    return w.reshape(KT, 128, c).transpose(1, 0, 2).reshape(128, KT * c)


def _make_in_maps(x, w_qkv, w_out, b_out):
    x = np.asarray(x, dtype=np.float32)
    w_qkv = np.asarray(w_qkv, dtype=np.float32)
    w_out = np.asarray(w_out, dtype=np.float32)
    b_out = np.asarray(b_out, dtype=np.float32)

    wqk = np.zeros((12, 128, KT * 128), dtype=np.float32)
    for g in range(6):
        wqk[2 * g] = _kslab(w_qkv[:, 128 * g:128 * (g + 1)])
        wqk[2 * g + 1] = _kslab(w_qkv[:, 768 + 128 * g:768 + 128 * (g + 1)])
    wqk_b = wqk.astype(BF16)
    wv65 = np.zeros((DIM, VW), dtype=np.float32)
    for h in range(HEADS):
        wv65[:, 65 * h:65 * h + 64] = w_qkv[:, 1536 + 64 * h:1536 + 64 * h + 64]
    wv_b = wv65.reshape(KT, 128, VW).astype(BF16)
    wo_b = w_out.astype(BF16)

    w_v = w_qkv[:, 1536:]
    w_k = w_qkv[:, 768:1536]
    vcls = x[:, 0, :] @ w_v
    kcls_f = x[:, 0, :] @ w_k

    vcrs = np.zeros((B, 1, VW), dtype=np.float32)
    kcls = np.zeros((B, 128, 12), dtype=np.float32)
    for b in range(B):
        vcrs[b, 0] = _v65(vcls[b])
        for g in range(6):
            kcls[b, 0:64, 2 * g] = kcls_f[b, 64 * 2 * g:64 * 2 * g + 64]
            kcls[b, 64:128, 2 * g + 1] = kcls_f[b, 64 * (2 * g + 1):64 * (2 * g + 1) + 64]

    ident = np.eye(128, dtype=BF16)
    bias_tiled = np.tile(b_out.reshape(1, DIM), (128, 1)).astype(BF16)
    in_maps = []
    for c in range(NCORES):
        xaug = np.zeros((B, TAUG, DIM), dtype=np.float32)
        xaug[:, 0, :] = x[:, 0, :]
        if c == 0:
            xaug[:, 1, :] = x[:, 1, :]
        L = ENDS[c] - STARTS[c]
        xaug[:, 2:2 + L, :] = x[:, STARTS[c]:ENDS[c], :]
        xaT = xaug.transpose(0, 2, 1).reshape(B, KT, 128, TAUG)
        m = _make_masks(c)
        mask4 = np.concatenate([m[0], m[1], m[0], m[1]], axis=1)
        in_maps.append({
            "xaT": xaT.astype(BF16),
            "wqk": wqk_b,
            "wv": wv_b,
            "w_out": wo_b,
            "b_out": bias_tiled,
            "mask4": mask4.astype(BF16),
            "ident": ident,
            "vc_rep": vcrs.astype(BF16),
            "kcls": kcls.astype(BF16),
        })
    return in_maps


def kernel(x, w_qkv, w_out, b_out):
    x_f = np.asarray(x, dtype=np.float64)
    w_qkv_f = np.asarray(w_qkv, dtype=np.float64)
    w_out_f = np.asarray(w_out, dtype=np.float64)
    b_out_f = np.asarray(b_out, dtype=np.float64)
    in_maps = _make_in_maps(x, w_qkv, w_out, b_out)

    from concourse.bass_utils import run_bass_kernel_spmd

    nc = _get_nc()
    res = run_bass_kernel_spmd(nc, in_maps, core_ids=list(range(NCORES))).results

    out = np.empty((B, N, DIM), dtype=np.float32)
    for c in range(NCORES):
        L = ENDS[c] - STARTS[c]
        out[:, STARTS[c]:ENDS[c], :] = res[c]["out_tokens"][:, :L, :]

    # CLS / tok1 rows: device partial sums over block keys + host-computed
    # contributions of the cls/tok1 keys themselves.
    w_q = w_qkv_f[:, 0:768]
    w_k = w_qkv_f[:, 768:1536]
    w_v = w_qkv_f[:, 1536:]
    for b in range(B):
        acc = np.zeros((HEADS, VW), dtype=np.float64)
        for c in range(NCORES):
            acc += res[c]["cls_part"][b].astype(np.float64)
            acc[:, 64::65] -= 256 - (ENDS[c] - STARTS[c])
        qc = x_f[b, 0] @ w_q
        qt = x_f[b, 1] @ w_q
        kc = x_f[b, 0] @ w_k
        kt = x_f[b, 1] @ w_k
        v65c = _v65(x_f[b, 0] @ w_v)
        v65t = _v65(x_f[b, 1] @ w_v)
        t1x = np.zeros((2 * HEADS, VW), dtype=np.float64)
        for h in range(HEADS):
            sl = slice(64 * h, 64 * h + 64)
            for j, qv in ((0, qc), (1, qt)):
                ec = np.exp(SCALE * np.dot(kc[sl], qv[sl]))
                et = np.exp(SCALE * np.dot(kt[sl], qv[sl]))
                t1x[2 * h + j] = ec * v65c + et * v65t
        acc += t1x[0::2]
        cls_flat = np.empty(INNER, dtype=np.float64)
        t1_flat = np.empty(INNER, dtype=np.float64)
        for h in range(HEADS):
            cls_flat[64 * h:64 * h + 64] = acc[h, 65 * h:65 * h + 64] / acc[h, 65 * h + 64]
            t1_flat[64 * h:64 * h + 64] = (t1x[2 * h + 1, 65 * h:65 * h + 64]
                                           / t1x[2 * h + 1, 65 * h + 64])
        out[b, 0, :] = (cls_flat @ w_out_f + b_out_f).astype(np.float32)
        out[b, 1, :] = (t1_flat @ w_out_f + b_out_f).astype(np.float32)
    return out


# revision 84
# speedup vs baseline: 1.3042x; 1.2525x over previous
import os
import sys

for _p in ("/opt/trn_rl_repo", "/root/.axon_site/_ro/trn_rl_repo"):
    if os.path.isdir(_p) and _p not in sys.path:
        sys.path.insert(0, _p)

import numpy as np
import ml_dtypes

BF16 = ml_dtypes.bfloat16

HEADS, D = 12, 64
WINDOW, SHIFT = 16, 1
SCALE = D ** -0.5
B, N, DIM = 2, 2049, 768
INNER = HEADS * D  # 768
TAUG = 258  # CLS slot + tok1/dummy slot + 256 block tokens
NCORES = 8
KT = DIM // 128  # 6
VW = HEADS * 65  # 780: per-head 64 v-cols + ones-col at 65h+64

STARTS = [2, 258, 514, 770, 1026, 1282, 1538, 1794]
ENDS = [258, 514, 770, 1026, 1282, 1538, 1794, 2049]

LEGACY_ATTN = True

_NC_CACHE = {}


def _build_nc():
    import concourse.bass as bass
    import concourse.bacc as bacc
    import concourse.mybir as mybir
    import concourse.tile as tile

    f32 = mybir.dt.float32
    bf16 = mybir.dt.bfloat16
    Exp = mybir.ActivationFunctionType.Exp
    Copy = mybir.ActivationFunctionType.Copy

    nc = bacc.Bacc(None, target_bir_lowering=False)

    xT_ext = nc.declare_dram_parameter("xaT", (B, KT, 128, TAUG), bf16, isOutput=False)
    # q/k weights as 12 column-slices [128, KT*128] in order q0,k0,q1,k1,...;
    # col-block k of slice = w[128k:128(k+1), cols].
    wqk_ext = nc.declare_dram_parameter("wqk", (12, 128, KT * 128), bf16, isOutput=False)
    wv_ext = nc.declare_dram_parameter("wv", (KT, 128, VW), bf16, isOutput=False)
    wout_ext = nc.declare_dram_parameter("w_out", (INNER, DIM), bf16, isOutput=False)
    bout_ext = nc.declare_dram_parameter("b_out", (128, DIM), bf16, isOutput=False)
    mask_ext = nc.declare_dram_parameter("mask4", (128, 512), bf16, isOutput=False)
    id_ext = nc.declare_dram_parameter("ident", (128, 128), bf16, isOutput=False)
    vcr_ext = nc.declare_dram_parameter("vc_rep", (B, 1, VW), bf16, isOutput=False)
    # zero-split CLS-key stationaries: col 2g has head 2g's k_cls in rows
    # 0:64 (zeros below); col 2g+1 has head 2g+1's k_cls in rows 64:128.
    kcls_ext = nc.declare_dram_parameter("kcls", (B, 128, 12), bf16, isOutput=False)
    out_ext = nc.declare_dram_parameter("out_tokens", (B, 256, DIM), bf16, isOutput=True)
    cls_ext = nc.declare_dram_parameter("cls_part", (B, HEADS, VW), f32, isOutput=True)

    with tile.TileContext(nc) as tc:
        with (
            tc.tile_pool(name="wpool", bufs=1) as wp,
            tc.tile_pool(name="fpool", bufs=2) as fp,
            tc.tile_pool(name="spool", bufs=6) as sp,
            tc.tile_pool(name="psA", bufs=2, space="PSUM") as psA,
            tc.tile_pool(name="psS", bufs=2, space="PSUM") as psS,
        ):
            # ---- input DMAs in compute order ----
            xT = [[None] * KT for _ in range(B)]
            for k in range(KT):
                t = fp.tile([128, TAUG], bf16, tag=f"xT0_{k}", name=f"xT0_{k}")
                nc.sync.dma_start(t[:], xT_ext[0, k])
                xT[0][k] = t[:]
            vcr, kcls_t = [None] * B, [None] * B
            vcr[0] = fp.tile([1, VW], bf16, tag="vcr0", name="vcr0")
            nc.sync.dma_start(vcr[0][:], vcr_ext[0])
            kcls_t[0] = wp.tile([128, 12], bf16, tag="kcls0", name="kcls0")
            nc.sync.dma_start(kcls_t[0][:], kcls_ext[0])
            wqk_t = []
            for s in range(12):
                t = wp.tile([128, KT * 128], bf16, tag=f"wqk{s}")
                nc.sync.dma_start(t[:], wqk_ext[s])
                wqk_t.append(t)
            wv_t = []
            for k in range(KT):
                t = wp.tile([128, VW], bf16, tag=f"wv{k}")
                nc.sync.dma_start(t[:], wv_ext[k])
                wv_t.append(t)
            mask4 = wp.tile([128, 512], bf16, tag="mask4")
            nc.sync.dma_start(mask4[:], mask_ext[:])
            ident = wp.tile([128, 128], bf16, tag="ident")
            nc.sync.dma_start(ident[:], id_ext[:])
            for k in range(KT):
                t = fp.tile([128, TAUG], bf16, tag=f"xT1_{k}", name=f"xT1_{k}")
                nc.sync.dma_start(t[:], xT_ext[1, k])
                xT[1][k] = t[:]
            vcr[1] = fp.tile([1, VW], bf16, tag="vcr1", name="vcr1")
            nc.sync.dma_start(vcr[1][:], vcr_ext[1])
            kcls_t[1] = wp.tile([128, 12], bf16, tag="kcls1", name="kcls1")
            nc.sync.dma_start(kcls_t[1][:], kcls_ext[1])
            wo_t = []
            for k in range(KT):
                t = wp.tile([128, DIM], bf16, tag=f"wo{k}")
                nc.sync.dma_start(t[:], wout_ext[k * 128:(k + 1) * 128, :])
                wo_t.append(t)
            bias_full = wp.tile([128, DIM], bf16, tag="bias_full")
            nc.sync.dma_start(bias_full[:], bout_ext[:])

            qT = [[None] * 6 for _ in range(B)]
            kTt = [[None] * 6 for _ in range(B)]
            vs = [[None, None] for _ in range(B)]
            att_s = [[fp.tile([128, INNER], bf16, tag=f"att{b}_{s}", name=f"att{b}_{s}")
                      for s in range(2)] for b in range(B)]
            aT = [[fp.tile([128, 256], bf16, tag=f"aT{b}_{i}", name=f"aT{b}_{i}")
                   for i in range(KT)] for b in range(B)]
            clspt = psS.tile([128, 512], f32, tag="cls", bufs=1, name="clspt")
            clsp = [clspt[:, 256 * b:256 * b + 256] for b in range(B)]

            def qk_unit(b, j, g, pref, dst):
                # j=0 -> q slice, j=1 -> k slice
                w = wqk_t[2 * g + j]
                ps = psA.tile([128, 512], f32, tag="big", name="ps")
                for k in range(KT):
                    nc.tensor.matmul(ps[:, 0:TAUG],
                                     w[:, 128 * k: 128 * (k + 1)],
                                     xT[b][k], start=(k == 0), stop=(k == KT - 1))
                t = fp.tile([128, TAUG], bf16, tag=f"{pref}T{b}_{g}", name=f"{pref}T{b}_{g}")
                nc.vector.tensor_copy(t[:], ps[:, 0:TAUG])
                dst[b][g] = t

            def v_unit(b, ti):
                vt = fp.tile([128, VW], bf16, tag=f"v{b}_{ti}", name=f"v{b}_{ti}")
                for c0, cw in ((0, 512), (512, VW - 512)):
                    pv = psA.tile([128, 512], f32, tag="big", name="pv")
                    for k in range(KT):
                        nc.tensor.matmul(pv[:, 0:cw],
                                         xT[b][k][:, 2 + 128 * ti: 2 + 128 * (ti + 1)],
                                         wv_t[k][:, c0: c0 + cw],
                                         start=(k == 0), stop=(k == KT - 1))
                    nc.vector.tensor_copy(vt[:, c0:c0 + cw], pv[:, 0:cw])
                nc.vector.memset(vt[:, 64:VW:65], 1.0)
                vs[b][ti] = vt

            def attn_head_legacy(b, h):
                g, p0 = h // 2, 64 * (h % 2)
                kk, qq = kTt[b][g], qT[b][g]
                hps = psS.tile([128, 512], f32, tag="hqk", name="hps")
                ecp2 = psS.tile([128, 512], f32, tag="ecp", bufs=1, name="ecp2")
                for half in range(2):
                    nc.tensor.matmul(ecp2[0:1, 256 * half:256 * half + 256],
                                     kk[p0:p0 + 64, 0:1], qq[p0:p0 + 64, 2:TAUG],
                                     start=True, stop=True, skip_group_check=True)
                ecr = sp.tile([1, 512], bf16, tag="ecr", name="ecr")
                nc.scalar.activation(ecr[:], ecp2[0:1, 0:512], Exp, scale=SCALE)
                # ecr col space: token index - 2
                for s in range(2):
                    q0 = 2 + 128 * s
                    nc.tensor.matmul(clsp[b][:, 12 * s + h:12 * s + h + 1],
                                     kk[p0:p0 + 64, q0:q0 + 128],
                                     qq[p0:p0 + 64, 0:1], start=True, stop=True,
                                     skip_group_check=True)
                pst = psS.tile([128, 256], f32, tag="hav", name="pst")
                for s in range(2):
                    q0 = 2 + 128 * s
                    nc.tensor.matmul(pst[:, 128 * s:128 * s + 128],
                                     kk[p0:p0 + 64, q0:q0 + 128],
                                     qq[p0:p0 + 64, q0:q0 + 128], start=True, stop=True,
                                     skip_group_check=True)
                prob = sp.tile([128, 256], bf16, tag="prob", name="prob")
                nc.scalar.activation(prob[:], pst[:], Exp, scale=SCALE)
                nc.vector.tensor_mul(prob[:], prob[:], mask4[:, 0:256])
                for s in range(2):
                    pc = TAUG + 65 * s
                    nc.tensor.matmul(hps[:, pc:pc + 65], prob[:, 128 * s:128 * s + 128],
                                     vs[b][s][:, 65 * h:65 * h + 65],
                                     start=True, stop=False, skip_group_check=True)
                    nc.tensor.matmul(hps[:, pc:pc + 65], ecr[0:1, 128 * s:128 * s + 128],
                                     vcr[b][0:1, 65 * h:65 * h + 65],
                                     start=False, stop=True, skip_group_check=True)
                rec = sp.tile([128, 2], f32, tag="rec", name="rec")
                nc.vector.reciprocal(rec[:], hps[:, TAUG + 64:TAUG + 130:65])
                for s in range(2):
                    pc = TAUG + 65 * s
                    nc.scalar.activation(att_s[b][s][:, 64 * h:64 * h + 64],
                                         hps[:, pc:pc + 64], Copy, scale=rec[:, s:s + 1])

            # ---- attention head-pair, split into front (QK/exp) and back
            # (AV/normalize) halves for software pipelining ----
            pair_state = {}

            def pair_front(b, g):
                kk, qq = kTt[b][g], qT[b][g]
                hq = [psS.tile([128, 256], f32, tag="hqk", name=f"hqk{h01}")
                      for h01 in range(2)]
                for h01 in range(2):
                    p0 = 64 * h01
                    h = 2 * g + h01
                    for s in range(2):
                        q0 = 2 + 128 * s
                        nc.tensor.matmul(hq[h01][:, 128 * s:128 * s + 128],
                                         kk[p0:p0 + 64, q0:q0 + 128],
                                         qq[p0:p0 + 64, q0:q0 + 128],
                                         start=True, stop=True, skip_group_check=True)
                        nc.tensor.matmul(clsp[b][:, 12 * s + h:12 * s + h + 1],
                                         kk[p0:p0 + 64, q0:q0 + 128],
                                         qq[p0:p0 + 64, 0:1], start=True, stop=True,
                                         skip_group_check=True)
                ecp = psS.tile([128, 512], f32, tag="ecp", bufs=1, name="ecp")
                for h01 in range(2):
                    p0 = 64 * h01
                    nc.tensor.matmul(ecp[0:1, 256 * h01:256 * h01 + 256],
                                     kk[p0:p0 + 64, 0:1],
                                     qq[p0:p0 + 64, 2:TAUG], start=True, stop=True,
                                     skip_group_check=True)
                ecr = sp.tile([1, 512], bf16, tag="ecr", name="ecr")
                nc.scalar.activation(ecr[:], ecp[0:1, :], Exp, scale=SCALE)
                prob2 = sp.tile([128, 512], bf16, tag="prob", name="prob2")
                for h01 in range(2):
                    c0 = 256 * h01
                    nc.scalar.activation(prob2[:, c0:c0 + 256], hq[h01][:],
                                         Exp, scale=SCALE)
                    nc.vector.tensor_mul(prob2[:, c0:c0 + 256], prob2[:, c0:c0 + 256],
                                         mask4[:, c0:c0 + 256])
                pair_state[(b, g)] = (prob2, ecr)

            def pair_back(b, g):
                prob2, ecr = pair_state.pop((b, g))
                hav = [psS.tile([128, 130], f32, tag="hav", name=f"hav{h01}")
                       for h01 in range(2)]
                for h01 in range(2):
                    h = 2 * g + h01
                    for s in range(2):
                        pc = 65 * s
                        nc.tensor.matmul(hav[h01][:, pc:pc + 65],
                                         prob2[:, 256 * h01 + 128 * s:256 * h01 + 128 * s + 128],
                                         vs[b][s][:, 65 * h:65 * h + 65],
                                         start=True, stop=False, skip_group_check=True)
                        nc.tensor.matmul(hav[h01][:, pc:pc + 65],
                                         ecr[0:1, 256 * h01 + 128 * s:256 * h01 + 128 * s + 128],
                                         vcr[b][0:1, 65 * h:65 * h + 65],
                                         start=False, stop=True, skip_group_check=True)
                for h01 in range(2):
                    h = 2 * g + h01
                    for s in range(2):
                        pc = 65 * s
                        rec = sp.tile([128, 1], f32, tag="rec", name="rec")
                        nc.vector.reciprocal(rec[:], hav[h01][:, pc + 64:pc + 65])
                        dst = att_s[b][s][:, 64 * h:64 * h + 64]
                        nc.scalar.activation(dst, hav[h01][:, pc:pc + 64], Copy,
                                             scale=rec[:, 0:1])

            def cls_unit(b):
                eccs = []
                for s in range(2):
                    E = sp.tile([128, HEADS], bf16, tag="ECC", name="E")
                    nc.scalar.activation(E[:], clsp[b][:, 12 * s:12 * (s + 1)], Exp, scale=SCALE)
                    eccs.append(E)
                clsA = psS.tile([128, 512], f32, tag="hqk", name="clsA")
                clsB = psS.tile([128, 512], f32, tag="hqk", name="clsB")
                for s in range(2):
                    nc.tensor.matmul(clsA[0:HEADS, :], eccs[s][:], vs[b][s][:, 0:512],
                                     start=(s == 0), stop=(s == 1), skip_group_check=True)
                    nc.tensor.matmul(clsB[0:HEADS, 0:VW - 512], eccs[s][:], vs[b][s][:, 512:VW],
                                     start=(s == 0), stop=(s == 1), skip_group_check=True)
                cls_sb = sp.tile([HEADS, VW], f32, tag="clssb", name="cls_sb")
                nc.vector.tensor_copy(cls_sb[:, 0:512], clsA[0:HEADS, :])
                nc.vector.tensor_copy(cls_sb[:, 512:VW], clsB[0:HEADS, 0:VW - 512])
                nc.sync.dma_start(cls_ext[b], cls_sb[:])

            def tr_unit(b, i):
                for s in range(2):
                    pt = psA.tile([128, 128], bf16, tag="big", name="pt")
                    nc.tensor.transpose(pt[:], att_s[b][s][:, 128 * i:128 * (i + 1)], ident[:])
                    nc.vector.tensor_copy(aT[b][i][:, 128 * s:128 * (s + 1)], pt[:])

            def proj_unit(b, ti):
                ot = fp.tile([128, DIM], bf16, tag=f"ot{b}_{ti}", name=f"ot{b}_{ti}")
                for c0, cw in ((0, 512), (512, 256)):
                    po = psA.tile([128, 512], f32, tag="big", name="po")
                    for i in range(KT):
                        nc.tensor.matmul(po[:, 0:cw], aT[b][i][:, 128 * ti:128 * (ti + 1)],
                                         wo_t[i][:, c0:c0 + cw],
                                         start=(i == 0), stop=(i == KT - 1))
                    nc.vector.tensor_add(ot[:, c0:c0 + cw], po[:, 0:cw],
                                         bias_full[:, c0:c0 + cw])
                    nc.sync.dma_start(
                        out_ext[b, 128 * ti:128 * (ti + 1), c0:c0 + cw],
                        ot[:, c0:c0 + cw])

            # ---- HAM warm-up: dummy matmuls on the first-arriving x tile
            # fill the DMA-wait window so phase 1 starts at full PE clock ----
            wps = psA.tile([128, 512], f32, tag="big", name="wps")
            for _ in range(40):
                nc.tensor.matmul(wps[:, 0:128], xT[0][0][:, 0:128],
                                 xT[0][0][:, 0:128], start=True, stop=True,
                                 skip_group_check=True)

            # ---- phase 1: QKV(b0) ----
            for g in range(6):
                qk_unit(0, 0, g, "q", qT)
                qk_unit(0, 1, g, "k", kTt)
            v_unit(0, 0)
            v_unit(0, 1)

            # ---- phase 2: attention(b0), QKV(b1) interleaved ----
            b1_units = ([lambda g=g, j=j: qk_unit(1, j, g, "qk"[j], (qT, kTt)[j])
                         for g in range(6) for j in range(2)]
                        + [lambda ti=ti: v_unit(1, ti) for ti in range(2)])
            for g in range(6):
                if LEGACY_ATTN:
                    attn_head_legacy(0, 2 * g)
                    attn_head_legacy(0, 2 * g + 1)
                else:
                    pair_front(0, g)
                    pair_back(0, g)
                tr_unit(0, g)
                for f in b1_units[14 * g // 6:14 * (g + 1) // 6]:
                    f()
            for f in b1_units[14:]:
                f()

            # ---- phase 3: attention(b1), b0 tail interleaved ----
            b0_tail = [lambda: cls_unit(0),
                       lambda: proj_unit(0, 0),
                       lambda: proj_unit(0, 1)]
            for g in range(6):
                if LEGACY_ATTN:
                    attn_head_legacy(1, 2 * g)
                    attn_head_legacy(1, 2 * g + 1)
                else:
                    pair_front(1, g)
                    pair_back(1, g)
                tr_unit(1, g)
                if g < 3:
                    b0_tail[g]()

            # ---- phase 4: tail for b1 ----
            cls_unit(1)
            proj_unit(1, 0)
            proj_unit(1, 1)

    nc.compile()
    return nc


def _get_nc():
    if "nc" not in _NC_CACHE:
        _NC_CACHE["nc"] = _build_nc()
    return _NC_CACHE["nc"]


def _make_masks(core):
    start = STARTS[core]
    masks = np.zeros((2, 128, 128), dtype=np.float32)
    for s in range(2):
        g = start + s * 128 + np.arange(128)
        real = g < 2049
        blk = (g - 2) // 16
        same = (blk[:, None] == blk[None, :]) & real[:, None] & real[None, :]
        masks[s] = same.astype(np.float32)
    return masks


def _v65(row768):
    out = np.zeros(VW, dtype=np.float64)
    for h in range(HEADS):
        out[65 * h:65 * h + 64] = row768[64 * h:64 * h + 64]
        out[65 * h + 64] = 1.0
    return out


def _kslab(w):
    # [768, C] -> [128, KT*C] with col-block k = w[128k:128(k+1), :]
    c = w.shape[1]
    return w.reshape(KT, 128, c).transpose(1, 0, 2).reshape(128, KT * c)


def _make_in_maps(x, w_qkv, w_out, b_out):
    x = np.asarray(x, dtype=np.float32)
    w_qkv = np.asarray(w_qkv, dtype=np.float32)
    w_out = np.asarray(w_out, dtype=np.float32)
    b_out = np.asarray(b_out, dtype=np.float32)

    wqk = np.zeros((12, 128, KT * 128), dtype=np.float32)
    for g in range(6):
        wqk[2 * g] = _kslab(w_qkv[:, 128 * g:128 * (g + 1)])
        wqk[2 * g + 1] = _kslab(w_qkv[:, 768 + 128 * g:768 + 128 * (g + 1)])
    wqk_b = wqk.astype(BF16)
    wv65 = np.zeros((DIM, VW), dtype=np.float32)
    for h in range(HEADS):
        wv65[:, 65 * h:65 * h + 64] = w_qkv[:, 1536 + 64 * h:1536 + 64 * h + 64]
    wv_b = wv65.reshape(KT, 128, VW).astype(BF16)
    wo_b = w_out.astype(BF16)

    w_v = w_qkv[:, 1536:]
    w_k = w_qkv[:, 768:1536]
    vcls = x[:, 0, :] @ w_v
    kcls_f = x[:, 0, :] @ w_k

    vcrs = np.zeros((B, 1, VW), dtype=np.float32)
    kcls = np.zeros((B, 128, 12), dtype=np.float32)
    for b in range(B):
        vcrs[b, 0] = _v65(vcls[b])
        for g in range(6):
            kcls[b, 0:64, 2 * g] = kcls_f[b, 64 * 2 * g:64 * 2 * g + 64]
            kcls[b, 64:128, 2 * g + 1] = kcls_f[b, 64 * (2 * g + 1):64 * (2 * g + 1) + 64]

    ident = np.eye(128, dtype=BF16)
    bias_tiled = np.tile(b_out.reshape(1, DIM), (128, 1)).astype(BF16)
    in_maps = []
    for c in range(NCORES):
        xaug = np.zeros((B, TAUG, DIM), dtype=np.float32)
        xaug[:, 0, :] = x[:, 0, :]
        if c == 0:
            xaug[:, 1, :] = x[:, 1, :]
        L = ENDS[c] - STARTS[c]
        xaug[:, 2:2 + L, :] = x[:, STARTS[c]:ENDS[c], :]
        xaT = xaug.transpose(0, 2, 1).reshape(B, KT, 128, TAUG)
        m = _make_masks(c)
        mask4 = np.concatenate([m[0], m[1], m[0], m[1]], axis=1)
        in_maps.append({
            "xaT": xaT.astype(BF16),
            "wqk": wqk_b,
            "wv": wv_b,
            "w_out": wo_b,
            "b_out": bias_tiled,
            "mask4": mask4.astype(BF16),
            "ident": ident,
            "vc_rep": vcrs.astype(BF16),
            "kcls": kcls.astype(BF16),
        })
    return in_maps


def kernel(x, w_qkv, w_out, b_out):
    x_f = np.asarray(x, dtype=np.float64)
    w_qkv_f = np.asarray(w_qkv, dtype=np.float64)
    w_out_f = np.asarray(w_out, dtype=np.float64)
    b_out_f = np.asarray(b_out, dtype=np.float64)
    in_maps = _make_in_maps(x, w_qkv, w_out, b_out)

    from concourse.bass_utils import run_bass_kernel_spmd

    nc = _get_nc()
    res = run_bass_kernel_spmd(nc, in_maps, core_ids=list(range(NCORES))).results

    out = np.empty((B, N, DIM), dtype=np.float32)
    for c in range(NCORES):
        L = ENDS[c] - STARTS[c]
        out[:, STARTS[c]:ENDS[c], :] = res[c]["out_tokens"][:, :L, :]

    # CLS / tok1 rows: device partial sums over block keys + host-computed
    # contributions of the cls/tok1 keys themselves.
    w_q = w_qkv_f[:, 0:768]
    w_k = w_qkv_f[:, 768:1536]
    w_v = w_qkv_f[:, 1536:]
    for b in range(B):
        acc = np.zeros((HEADS, VW), dtype=np.float64)
        for c in range(NCORES):
            acc += res[c]["cls_part"][b].astype(np.float64)
            acc[:, 64::65] -= 256 - (ENDS[c] - STARTS[c])
        qc = x_f[b, 0] @ w_q
        qt = x_f[b, 1] @ w_q
        kc = x_f[b, 0] @ w_k
        kt = x_f[b, 1] @ w_k
        v65c = _v65(x_f[b, 0] @ w_v)
        v65t = _v65(x_f[b, 1] @ w_v)
        t1x = np.zeros((2 * HEADS, VW), dtype=np.float64)
        for h in range(HEADS):
            sl = slice(64 * h, 64 * h + 64)
            for j, qv in ((0, qc), (1, qt)):
                ec = np.exp(SCALE * np.dot(kc[sl], qv[sl]))
                et = np.exp(SCALE * np.dot(kt[sl], qv[sl]))
                t1x[2 * h + j] = ec * v65c + et * v65t
        acc += t1x[0::2]
        cls_flat = np.empty(INNER, dtype=np.float64)
        t1_flat = np.empty(INNER, dtype=np.float64)
        for h in range(HEADS):
            cls_flat[64 * h:64 * h + 64] = acc[h, 65 * h:65 * h + 64] / acc[h, 65 * h + 64]
            t1_flat[64 * h:64 * h + 64] = (t1x[2 * h + 1, 65 * h:65 * h + 64]
                                           / t1x[2 * h + 1, 65 * h + 64])
        out[b, 0, :] = (cls_flat @ w_out_f + b_out_f).astype(np.float32)
        out[b, 1, :] = (t1_flat @ w_out_f + b_out_f).astype(np.float32)
    return out
